# revision 10
# baseline (speedup 1.0000x reference)
"""AspectAttention Trainium2 kernel (8 NeuronCores, batch-parallel, fp8).

out = tok * (1 + softmax_S(tanh(cat(tok, mean_A(asp)) @ W + b) @ v))

Sharding: data-parallel over batch B=16 -> 2 batches per core. Softmax is
per-(batch) row over S, so no cross-core communication is needed.

Per-core math (concat split): E^T = tanh(W1^T @ X^T + bias), where
bias = (mean_A(asp) @ W2 + b) is per-batch; scores = v^T @ E^T;
weights = softmax(scores); out = X * (1 + weights).

The heavy matmul runs in fp8e4m3 with perf_mode=DoubleRow (2 k-chunks per
matmul). W1 is pre-scaled by 64 into fp8 to stay out of the subnormal
range; the tanh activation un-scales with scale=1/64. Empirically (exact
problem inputs) this lands at rel err ~1.7e-3 vs the 2e-2 gate.
"""

from contextlib import ExitStack

import numpy as np

import concourse.bass as bass
import concourse.mybir as mybir
import concourse.tile as tile
from concourse import bacc, bass_isa
from concourse.bass_utils import run_bass_kernel_spmd
from concourse.masks import make_identity

B, S, H, A = 16, 2048, 1024, 8
NCORES = 8
BPC = B // NCORES          # batches per core = 2
T = BPC * S                # tokens per core = 4096
NT = T // 128              # 32 token-128 tiles per core
NT512 = T // 512           # 8 token-512 tiles per core
KC = H // 128              # 8 contraction chunks
KP = KC // 2               # 4 double-row k-pairs
MC = H // 128              # 8 output-dim chunks
WSCALE = 64.0              # W1 fp8 pre-scale

F32 = mybir.dt.float32
F16 = mybir.dt.float16
F8 = mybir.dt.float8e4
ALU = mybir.AluOpType
AF = mybir.ActivationFunctionType
AX = mybir.AxisListType
DR = mybir.MatmulPerfMode.DoubleRow


def _emit(ctx: ExitStack, tc: "tile.TileContext", out, tok, asp, W, bvec, vvec):
    nc = tc.nc

    const = ctx.enter_context(tc.tile_pool(name="const", bufs=1))
    xres = ctx.enter_context(tc.tile_pool(name="xres", bufs=26))
    xtp = ctx.enter_context(tc.tile_pool(name="xtp", bufs=2))
    xbp = ctx.enter_context(tc.tile_pool(name="xbp", bufs=8))
    ep = ctx.enter_context(tc.tile_pool(name="ep", bufs=9))
    w2p = ctx.enter_context(tc.tile_pool(name="w2p", bufs=3))
    smp = ctx.enter_context(tc.tile_pool(name="smp", bufs=1))

    # PSUM budget is 8 banks, bank-granular: mm 3 + tp 2 + vd 1 + sm 2
    mm_ps = ctx.enter_context(tc.tile_pool(name="mm_ps", bufs=3, space="PSUM"))
    tp_ps = ctx.enter_context(tc.tile_pool(name="tp_ps", bufs=2, space="PSUM"))
    vd_ps = ctx.enter_context(tc.tile_pool(name="vd_ps", bufs=1, space="PSUM"))
    sm_ps = ctx.enter_context(tc.tile_pool(name="sm_ps", bufs=1, space="PSUM"))

    s_sb = const.tile([128, NT], F32)   # per-token-tile (1 + weight) scales

    # HAM warmup: dummy matmuls keep the PE busy from t=0 while the first
    # DMAs land, so the clock is at 8/8 when real work starts. The result
    # is copied (on gpsimd, off the DVE critical path) into s_sb[:, 0:1],
    # which is rewritten by the softmax scales before any consumer reads.
    warm = const.tile([128, 512], F8)
    nc.vector.memset(warm[:], 0.0)
    wps = mm_ps.tile([128, 512], F32, tag="mm", name="warm_ps")
    for r in range(22):
        nc.tensor.matmul(wps[:], warm[:, 0:128], warm[:],
                         start=True, stop=True, skip_group_check=True)
    nc.vector.tensor_copy(out=s_sb[:, 0:1], in_=wps[:, 0:1])

    # ---- constants / small inputs -------------------------------------
    ident = const.tile([128, 128], F32)
    make_identity(nc, ident[:])
    identh = const.tile([128, 128], F16)
    make_identity(nc, identh[:])
    # sel[p, f] = 1/8 iff p // 8 == f : mean-pool both batches via one matmul
    sel = const.tile([16, BPC], F32)
    nc.gpsimd.memset(sel[:], 0.125)
    nc.gpsimd.affine_select(
        out=sel[:], in_=sel[:], compare_op=ALU.is_ge, fill=0.0,
        base=0, pattern=[[-8, BPC]], channel_multiplier=1)
    nc.gpsimd.affine_select(
        out=sel[:], in_=sel[:], compare_op=ALU.is_ge, fill=0.0,
        base=7, pattern=[[8, BPC]], channel_multiplier=-1)

    v_sb = const.tile([128, MC], F16)            # v[m*128+p] at [p, m]
    v_stg = const.tile([128, MC], F32)
    nc.scalar.dma_start(v_stg[:], vvec.rearrange("(m p) -> p m", p=128))
    nc.vector.tensor_copy(out=v_sb[:], in_=v_stg[:])
    b_sb = const.tile([128, MC], F32)
    nc.scalar.dma_start(b_sb[:], bvec.rearrange("(m p) -> p m", p=128))

    asp_sb = const.tile([16, H], F32)            # (b, a) on partitions
    nc.scalar.dma_start(asp_sb[:], asp.rearrange("b a h -> (b a) h"))

    # ---- main pipeline ------------------------------------------------
    tok_t = tok.rearrange("(n p) h -> n p h", p=128)
    out_t = out.rearrange("(n p) h -> n p h", p=128)

    x_nat = {}          # n -> resident [128, H] f32 tile
    sc_rows = {}        # t -> [1, 512] score row awaiting transpose
    sT = {}             # bb -> [128, 16] transposed scores
    for bb in range(BPC):
        sT[bb] = smp.tile([128, 16], F32, tag=f"sT{bb}", name=f"sT{bb}")

    LOAD_Q = [nc.sync, nc.scalar, nc.sync, nc.scalar]

    def load_x(t):
        for j in range(4):
            n = 4 * t + j
            xt_ = xres.tile([128, H], F32, tag="x", name=f"x{n}")
            x_nat[n] = xt_
            LOAD_Q[j].dma_start(xt_[:], tok_t[n])

    def emit_score_transposes(t):
        bb = t // (NT512 // BPC)
        row = sc_rows.pop(t)
        for jj in range(4):
            col = 4 * (t % (NT512 // BPC)) + jj
            ps = tp_ps.tile([128, 1], F32, tag="tp")
            nc.tensor.transpose(
                ps[:], row[0:1, jj * 128 : (jj + 1) * 128], ident[0:1, 0:1])
            nc.vector.tensor_copy(out=sT[bb][:, col : col + 1], in_=ps[:])

    def emit_softmax_and_scales(bb):
        # softmax over the transposed [128, 16] score block, then the
        # per-token scale s = 1 + exp(x - max)/sum, then out = X * s
        stile = sT[bb]
        pmax = smp.tile([128, 1], F32, tag="pmax", name=f"pmax{bb}")
        nc.vector.tensor_reduce(pmax[:], stile[:], axis=AX.X, op=ALU.max)
        gmax = smp.tile([128, 1], F32, tag="gmax", name=f"gmax{bb}")
        nc.gpsimd.partition_all_reduce(
            gmax[:], pmax[:], channels=128, reduce_op=bass_isa.ReduceOp.max)
        negmax = smp.tile([128, 1], F32, tag="negmax", name=f"negmax{bb}")
        nc.vector.tensor_scalar(negmax[:], gmax[:], -1.0, None, op0=ALU.mult)
        acc = smp.tile([128, 1], F32, tag="acc", name=f"acc{bb}")
        sl = s_sb[:, bb * 16 : (bb + 1) * 16]
        nc.scalar.activation(sl, stile[:], AF.Exp, bias=negmax[:],
                             accum_out=acc[:])
        gsum = smp.tile([128, 1], F32, tag="gsum", name=f"gsum{bb}")
        nc.gpsimd.partition_all_reduce(
            gsum[:], acc[:], channels=128, reduce_op=bass_isa.ReduceOp.add)
        rc = smp.tile([128, 1], F32, tag="rc", name=f"rc{bb}")
        nc.vector.reciprocal(rc[:], gsum[:])
        nc.vector.tensor_scalar(sl, sl, rc[:], 1.0, op0=ALU.mult, op1=ALU.add)

    def emit_scales(bb, js, queues, use_act=False):
        for i, j in enumerate(js):
            n = bb * (NT // BPC) + j
            if use_act and j % 2 == 1:
                nc.scalar.mul(x_nat[n][:], x_nat[n][:], s_sb[:, n : n + 1])
            else:
                nc.vector.tensor_scalar(
                    x_nat[n][:], x_nat[n][:], s_sb[:, n : n + 1], None,
                    op0=ALU.mult)
            queues[i % len(queues)].dma_start(out_t[n], x_nat[n][:])

    pending_fin = None
    pending_scale = None
    load_x(0)
    # W1 staged f32 on the two load queues; scaled+cast to fp8 by DVE
    # (cast emission happens inside block 0, after the xb casts, to keep
    # the in-order DVE queue from stalling the first transposes)
    w1_sb = const.tile([128, KC, H], F8)         # 64*W1[k*128+p, m] at [p, k, m]
    w1_src = W[0:H, :].rearrange("(k p) m -> p k m", p=128)
    w1stg = {}
    for k in range(KC):
        stg = w2p.tile([128, H], F32, tag="w1stg", bufs=6, name=f"w1stg{k}")
        nc.sync.dma_start(stg[:], w1_src[:, k])
        w1stg[k] = stg
    holder = {}

    def emit_w1_casts():
        for k in range(KC):
            nc.vector.tensor_scalar(
                w1_sb[:, k, :], w1stg[k][:], WSCALE, None, op0=ALU.mult)

    def emit_bias_head():
        # ---- aspect mean via sel matmul: mean[b, h] on 2 partitions ----
        # the two [2, 512] psum tiles are reused: first for the mean
        # matmuls, then as the bias accumulators.
        bias_ps = [sm_ps.tile([BPC, 512], F32, tag=f"smb{h}", name=f"bias_ps{h}")
                   for h in range(2)]
        holder['bias_ps'] = bias_ps
        mean_sb = const.tile([BPC, H], F32)
        for half in range(2):
            nc.tensor.matmul(
                bias_ps[half][:], sel[:],
                asp_sb[:, half * 512 : (half + 1) * 512],
                start=True, stop=True)
            nc.scalar.copy(mean_sb[:, half * 512 : (half + 1) * 512],
                           bias_ps[half][:])

        # transpose mean into [h_in partitions, k, batch]
        meanT = const.tile([128, KC, BPC], F32)
        holder['meanT'] = meanT
        for k in range(KC):
            ps = tp_ps.tile([128, BPC], F32, tag="tp")
            nc.tensor.transpose(
                ps[:], mean_sb[:, k * 128 : (k + 1) * 128], ident[0:BPC, 0:BPC])
            nc.scalar.copy(meanT[:, k, :], ps[:])

    def emit_bias_k(k):
        # bias[b, m] += meanT[k]^T @ W2[k-chunk]: f32 matmuls streamed as
        # each staged W2 chunk lands; cheap startup filler for the PE
        bias_ps = holder['bias_ps']
        w2stg = w2p.tile([128, H], F32, tag="w2stg", bufs=3, name=f"w2stg{k}")
        nc.scalar.dma_start(w2stg[:], W[H + k * 128 : H + (k + 1) * 128, :])
        for half in range(2):
            nc.tensor.matmul(
                bias_ps[half][:], holder['meanT'][:, k, :],
                w2stg[:, half * 512 : (half + 1) * 512],
                start=(k == 0), stop=(k == KC - 1))

    def emit_bias_finish():
        bias_ps = holder['bias_ps']
        bias_sb = const.tile([BPC, H], F32)
        for half in range(2):
            nc.scalar.copy(bias_sb[:, half * 512 : (half + 1) * 512],
                           bias_ps[half][:])

        # transpose bias rows into biasT[m][p, batch] and add b
        biasT = const.tile([128, MC, BPC], F32)
        holder['biasT'] = biasT
        for m in range(MC):
            ps = tp_ps.tile([128, BPC], F32, tag="tp")
            nc.tensor.transpose(
                ps[:], bias_sb[:, m * 128 : (m + 1) * 128], ident[0:BPC, 0:BPC])
            nc.vector.tensor_scalar(
                biasT[:, m, :], ps[:], b_sb[:, m : m + 1], None, op0=ALU.add)

    def emit_group(t, m, mm):
        # E^T m-group: 4 double-row fp8 matmuls (k-pairs)
        # group 0 reads k-pairs in reverse: its first matmul then depends
        # on the final transpose copy, which stops the scheduler from
        # interleaving transposes into the matmul stream
        xT = holder['xT']
        kps = list(reversed(range(KP))) if (m == 0 and t > 0) else list(range(KP))
        for i, kp in enumerate(kps):
            nc.tensor.matmul(
                mm[:],
                w1_sb[:, 2 * kp : 2 * kp + 2, m * 128 : (m + 1) * 128],
                xT[:, 2 * kp : 2 * kp + 2, :],
                start=(i == 0), stop=(i == KP - 1), perf_mode=DR)

    for t in range(NT512):
        bb = t // (NT512 // BPC)
        if t > 0 and t + 2 < NT512:
            load_x(t + 2)

        # cast to fp8, transpose X block -> [h_in part, k, 512 tokens].
        # k-outer so xT chunks complete in the order the matmuls read them.
        xT = xtp.tile([128, KC, 512], F8, tag="xT")
        holder['xT'] = xT
        xbs = []
        for j in range(4):
            n = 4 * t + j
            xb = xbp.tile([128, H], F16, tag="xb", name=f"xb{n}")
            nc.gpsimd.tensor_copy(out=xb[:], in_=x_nat[n][:])
            xbs.append(xb)
        for kp in range(KP):
            ps = tp_ps.tile([128, 1024], F16, tag="tp")
            for kk in range(2):
                k = 2 * kp + kk
                for j in range(4):
                    nc.tensor.transpose(
                        ps[:, kk * 512 + j * 128 : kk * 512 + (j + 1) * 128],
                        xbs[j][:, k * 128 : (k + 1) * 128], identh[:])
            nc.vector.tensor_copy(out=xT[:, 2 * kp : 2 * kp + 2, :], in_=ps[:])

        if t == 0:
            emit_w1_casts()

        # previous block's score-row transposes (its ACT copy has finished
        # during our transpose phase, so the PE does not stall)
        if t - 1 in sc_rows:
            emit_score_transposes(t - 1)

        # E^T = tanh((64*W1)^T @ X^T / 64 + bias); scores += v^T @ E^T
        # (vdot for group m is emitted after matmul group m+1 so the PE
        # never waits on the tanh producing e[m])
        sc_ps = vd_ps.tile([1, 512], F32, tag="vd")
        e_tiles = []

        def tanh_and_vdot(m, mm):
            e = ep.tile([128, 512], F16, tag="e")
            nc.scalar.activation(e[:], mm[:], AF.Tanh,
                                 bias=holder['biasT'][:, m, bb : bb + 1],
                                 scale=1.0 / WSCALE)
            e_tiles.append((m, e))

        if t == 0:
            # block 0: bias-path f32 matmuls interleave with a k-pair-outer
            # sweep over three m-groups, in DMA arrival order, so the PE
            # always has work while W1/W2 stream from HBM; m3-7 run
            # k-pair-inner afterwards (W1 fully resident by then).
            emit_bias_head()
            mms = [mm_ps.tile([128, 512], F32, tag="mm", name=f"mm0_{m}")
                   for m in range(3)]
            for kp in range(KP):
                emit_bias_k(2 * kp)
                emit_bias_k(2 * kp + 1)
                for m in range(3):
                    nc.tensor.matmul(
                        mms[m][:],
                        w1_sb[:, 2 * kp : 2 * kp + 2, m * 128 : (m + 1) * 128],
                        xT[:, 2 * kp : 2 * kp + 2, :],
                        start=(kp == 0), stop=(kp == KP - 1), perf_mode=DR)
            emit_bias_finish()
            load_x(1)
            load_x(2)
            for m in range(3):
                tanh_and_vdot(m, mms[m])
            for m in range(3, MC):
                mm = mm_ps.tile([128, 512], F32, tag="mm")
                emit_group(t, m, mm)
                tanh_and_vdot(m, mm)
        else:
            for m in range(MC):
                mm = mm_ps.tile([128, 512], F32, tag="mm")
                emit_group(t, m, mm)
                tanh_and_vdot(m, mm)

        for pm, pe_t in e_tiles:
            nc.tensor.matmul(
                sc_ps[:], v_sb[:, pm : pm + 1], pe_t[:],
                start=(pm == 0), stop=(pm == MC - 1), skip_group_check=True)
        row = smp.tile([1, 512], F32, tag="scrow", bufs=2, name=f"row{t}")
        nc.scalar.copy(row[:], sc_ps[:])
        sc_rows[t] = row

        # finalize work is emitted AFTER the matmul section: the DVE queue
        # is in-order, and scales queued ahead of the next block's work
        # were stalling the PE at every finalize iteration
        if pending_fin is not None:
            emit_softmax_and_scales(pending_fin)
            emit_scales(pending_fin, range(0, 8), [nc.sync, nc.scalar],
                        use_act=True)
            pending_scale = pending_fin
            pending_fin = None
        elif pending_scale is not None:
            emit_scales(pending_scale, range(8, 16), [nc.sync, nc.scalar],
                        use_act=True)
            pending_scale = None

        if t % (NT512 // BPC) == (NT512 // BPC) - 1:
            pending_fin = bb

    # tail: last block's score transposes + last batch softmax/scales;
    # stores fan out over three queues to saturate HBM write bandwidth
    if pending_scale is not None:
        emit_scales(pending_scale, range(8, 16), [nc.sync, nc.scalar])
    emit_score_transposes(NT512 - 1)
    emit_softmax_and_scales(pending_fin)
    emit_scales(pending_fin, range(0, 16), [nc.sync, nc.scalar])


_CACHE = {}


def _build():
    if "nc" in _CACHE:
        return _CACHE["nc"]
    nc = bacc.Bacc("TRN2", target_bir_lowering=False, debug=False,
                   num_devices=NCORES)
    tok = nc.dram_tensor("tok", [T, H], F32, kind="ExternalInput").ap()
    asp = nc.dram_tensor("asp", [BPC, A, H], F32, kind="ExternalInput").ap()
    W_ = nc.dram_tensor("W", [2 * H, H], F32, kind="ExternalInput").ap()
    b_ = nc.dram_tensor("b", [H], F32, kind="ExternalInput").ap()
    v_ = nc.dram_tensor("v", [H], F32, kind="ExternalInput").ap()
    outp = nc.dram_tensor("out", [T, H], F32, kind="ExternalOutput").ap()

    with tile.TileContext(nc) as tc:
        with ExitStack() as ctx:
            _emit(ctx, tc, outp, tok, asp, W_, b_, v_)
    nc.compile()
    _CACHE["nc"] = nc
    return nc


def make_in_maps(token_embeddings, aspect_embedding, W, b, v):
    in_maps = []
    for c in range(NCORES):
        in_maps.append({
            "tok": np.ascontiguousarray(
                token_embeddings[BPC * c : BPC * (c + 1)].reshape(T, H)),
            "asp": np.ascontiguousarray(
                aspect_embedding[BPC * c : BPC * (c + 1)]),
            "W": W, "b": b, "v": v,
        })
    return in_maps


def kernel(token_embeddings, aspect_embedding, W, b, v):
    token_embeddings = np.asarray(token_embeddings, dtype=np.float32)
    aspect_embedding = np.asarray(aspect_embedding, dtype=np.float32)
    W = np.asarray(W, dtype=np.float32)
    b = np.asarray(b, dtype=np.float32)
    v = np.asarray(v, dtype=np.float32)

    nc = _build()
    in_maps = make_in_maps(token_embeddings, aspect_embedding, W, b, v)
    res = run_bass_kernel_spmd(nc, in_maps, core_ids=list(range(NCORES)))
    return np.concatenate(
        [res.results[c]["out"].reshape(BPC, S, H) for c in range(NCORES)], axis=0)


# revision 11
# speedup vs baseline: 1.0054x; 1.0054x over previous
"""AspectAttention Trainium2 kernel (8 NeuronCores, batch-parallel, fp8).

out = tok * (1 + softmax_S(tanh(cat(tok, mean_A(asp)) @ W + b) @ v))

Sharding: data-parallel over batch B=16 -> 2 batches per core. Softmax is
per-(batch) row over S, so no cross-core communication is needed.

Per-core math (concat split): E^T = tanh(W1^T @ X^T + bias), where
bias = (mean_A(asp) @ W2 + b) is per-batch; scores = v^T @ E^T;
weights = softmax(scores); out = X * (1 + weights).

The heavy matmul runs in fp8e4m3 with perf_mode=DoubleRow (2 k-chunks per
matmul). W1 is pre-scaled by 64 into fp8 to stay out of the subnormal
range; the tanh activation un-scales with scale=1/64. Empirically (exact
problem inputs) this lands at rel err ~1.7e-3 vs the 2e-2 gate.
"""

from contextlib import ExitStack

import numpy as np

import concourse.bass as bass
import concourse.mybir as mybir
import concourse.tile as tile
from concourse import bacc, bass_isa
from concourse.bass_utils import run_bass_kernel_spmd
from concourse.masks import make_identity

B, S, H, A = 16, 2048, 1024, 8
NCORES = 8
BPC = B // NCORES          # batches per core = 2
T = BPC * S                # tokens per core = 4096
NT = T // 128              # 32 token-128 tiles per core
NT512 = T // 512           # 8 token-512 tiles per core
KC = H // 128              # 8 contraction chunks
KP = KC // 2               # 4 double-row k-pairs
MC = H // 128              # 8 output-dim chunks
WSCALE = 64.0              # W1 fp8 pre-scale

F32 = mybir.dt.float32
F16 = mybir.dt.float16
F8 = mybir.dt.float8e4
ALU = mybir.AluOpType
AF = mybir.ActivationFunctionType
AX = mybir.AxisListType
DR = mybir.MatmulPerfMode.DoubleRow


def _emit(ctx: ExitStack, tc: "tile.TileContext", out, tok, asp, W, bvec, vvec):
    nc = tc.nc

    const = ctx.enter_context(tc.tile_pool(name="const", bufs=1))
    xres = ctx.enter_context(tc.tile_pool(name="xres", bufs=26))
    xtp = ctx.enter_context(tc.tile_pool(name="xtp", bufs=2))
    xbp = ctx.enter_context(tc.tile_pool(name="xbp", bufs=8))
    ep = ctx.enter_context(tc.tile_pool(name="ep", bufs=9))
    w2p = ctx.enter_context(tc.tile_pool(name="w2p", bufs=3))
    smp = ctx.enter_context(tc.tile_pool(name="smp", bufs=1))

    # PSUM budget is 8 banks, bank-granular: mm 3 + tp 2 + vd 1 + sm 2
    mm_ps = ctx.enter_context(tc.tile_pool(name="mm_ps", bufs=3, space="PSUM"))
    tp_ps = ctx.enter_context(tc.tile_pool(name="tp_ps", bufs=2, space="PSUM"))
    vd_ps = ctx.enter_context(tc.tile_pool(name="vd_ps", bufs=1, space="PSUM"))
    sm_ps = ctx.enter_context(tc.tile_pool(name="sm_ps", bufs=1, space="PSUM"))

    s_sb = const.tile([128, NT], F32)   # per-token-tile (1 + weight) scales

    # HAM warmup: dummy matmuls keep the PE busy from t=0 while the first
    # DMAs land, so the clock is at 8/8 when real work starts. The result
    # is copied (on gpsimd, off the DVE critical path) into s_sb[:, 0:1],
    # which is rewritten by the softmax scales before any consumer reads.
    warm = const.tile([128, 512], F8)
    nc.vector.memset(warm[:], 0.0)
    wps = mm_ps.tile([128, 512], F32, tag="mm", name="warm_ps")
    for r in range(22):
        nc.tensor.matmul(wps[:], warm[:, 0:128], warm[:],
                         start=True, stop=True, skip_group_check=True)
    nc.vector.tensor_copy(out=s_sb[:, 0:1], in_=wps[:, 0:1])

    # ---- constants / small inputs -------------------------------------
    ident = const.tile([128, 128], F32)
    make_identity(nc, ident[:])
    identh = const.tile([128, 128], F16)
    make_identity(nc, identh[:])
    # sel[p, f] = 1/8 iff p // 8 == f : mean-pool both batches via one matmul
    sel = const.tile([16, BPC], F32)
    nc.gpsimd.memset(sel[:], 0.125)
    nc.gpsimd.affine_select(
        out=sel[:], in_=sel[:], compare_op=ALU.is_ge, fill=0.0,
        base=0, pattern=[[-8, BPC]], channel_multiplier=1)
    nc.gpsimd.affine_select(
        out=sel[:], in_=sel[:], compare_op=ALU.is_ge, fill=0.0,
        base=7, pattern=[[8, BPC]], channel_multiplier=-1)

    v_sb = const.tile([128, MC], F16)            # v[m*128+p] at [p, m]
    v_stg = const.tile([128, MC], F32)
    nc.scalar.dma_start(v_stg[:], vvec.rearrange("(m p) -> p m", p=128))
    nc.vector.tensor_copy(out=v_sb[:], in_=v_stg[:])
    b_sb = const.tile([128, MC], F32)
    nc.scalar.dma_start(b_sb[:], bvec.rearrange("(m p) -> p m", p=128))

    asp_sb = const.tile([16, H], F32)            # (b, a) on partitions
    nc.scalar.dma_start(asp_sb[:], asp.rearrange("b a h -> (b a) h"))

    # ---- main pipeline ------------------------------------------------
    tok_t = tok.rearrange("(n p) h -> n p h", p=128)
    out_t = out.rearrange("(n p) h -> n p h", p=128)

    x_nat = {}          # n -> resident [128, H] f32 tile
    sc_rows = {}        # t -> [1, 512] score row awaiting transpose
    sT = {}             # bb -> [128, 16] transposed scores
    for bb in range(BPC):
        sT[bb] = smp.tile([128, 16], F32, tag=f"sT{bb}", name=f"sT{bb}")

    LOAD_Q = [nc.sync, nc.gpsimd, nc.sync, nc.gpsimd]

    def load_x(t):
        for j in range(4):
            n = 4 * t + j
            xt_ = xres.tile([128, H], F32, tag="x", name=f"x{n}")
            x_nat[n] = xt_
            LOAD_Q[j].dma_start(xt_[:], tok_t[n])

    def emit_score_transposes(t):
        bb = t // (NT512 // BPC)
        row = sc_rows.pop(t)
        for jj in range(4):
            col = 4 * (t % (NT512 // BPC)) + jj
            ps = tp_ps.tile([128, 1], F32, tag="tp")
            nc.tensor.transpose(
                ps[:], row[0:1, jj * 128 : (jj + 1) * 128], ident[0:1, 0:1])
            nc.vector.tensor_copy(out=sT[bb][:, col : col + 1], in_=ps[:])

    def emit_softmax_and_scales(bb):
        # softmax over the transposed [128, 16] score block, then the
        # per-token scale s = 1 + exp(x - max)/sum, then out = X * s
        stile = sT[bb]
        pmax = smp.tile([128, 1], F32, tag="pmax", name=f"pmax{bb}")
        nc.vector.tensor_reduce(pmax[:], stile[:], axis=AX.X, op=ALU.max)
        gmax = smp.tile([128, 1], F32, tag="gmax", name=f"gmax{bb}")
        nc.gpsimd.partition_all_reduce(
            gmax[:], pmax[:], channels=128, reduce_op=bass_isa.ReduceOp.max)
        negmax = smp.tile([128, 1], F32, tag="negmax", name=f"negmax{bb}")
        nc.vector.tensor_scalar(negmax[:], gmax[:], -1.0, None, op0=ALU.mult)
        acc = smp.tile([128, 1], F32, tag="acc", name=f"acc{bb}")
        sl = s_sb[:, bb * 16 : (bb + 1) * 16]
        nc.scalar.activation(sl, stile[:], AF.Exp, bias=negmax[:],
                             accum_out=acc[:])
        gsum = smp.tile([128, 1], F32, tag="gsum", name=f"gsum{bb}")
        nc.gpsimd.partition_all_reduce(
            gsum[:], acc[:], channels=128, reduce_op=bass_isa.ReduceOp.add)
        rc = smp.tile([128, 1], F32, tag="rc", name=f"rc{bb}")
        nc.vector.reciprocal(rc[:], gsum[:])
        nc.vector.tensor_scalar(sl, sl, rc[:], 1.0, op0=ALU.mult, op1=ALU.add)

    def emit_scales(bb, js, queues, use_act=False):
        for i, j in enumerate(js):
            n = bb * (NT // BPC) + j
            if use_act and i % 2 == 1:
                nc.scalar.mul(x_nat[n][:], x_nat[n][:], s_sb[:, n : n + 1])
            else:
                nc.vector.tensor_scalar(
                    x_nat[n][:], x_nat[n][:], s_sb[:, n : n + 1], None,
                    op0=ALU.mult)
            queues[i % len(queues)].dma_start(out_t[n], x_nat[n][:])

    pending_fin = None
    pending_scale = None
    load_x(0)
    # W1 staged f32 on the two load queues; scaled+cast to fp8 by DVE
    # (cast emission happens inside block 0, after the xb casts, to keep
    # the in-order DVE queue from stalling the first transposes)
    w1_sb = const.tile([128, KC, H], F8)         # 64*W1[k*128+p, m] at [p, k, m]
    w1_src = W[0:H, :].rearrange("(k p) m -> p k m", p=128)
    w1stg = {}
    for k in range(KC):
        stg = w2p.tile([128, H], F32, tag="w1stg", bufs=6, name=f"w1stg{k}")
        nc.sync.dma_start(stg[:], w1_src[:, k])
        w1stg[k] = stg
    holder = {}

    def emit_w1_casts():
        for k in range(KC):
            nc.vector.tensor_scalar(
                w1_sb[:, k, :], w1stg[k][:], WSCALE, None, op0=ALU.mult)

    def emit_bias_head():
        # ---- aspect mean via sel matmul: mean[b, h] on 2 partitions ----
        # the two [2, 512] psum tiles are reused: first for the mean
        # matmuls, then as the bias accumulators.
        bias_ps = [sm_ps.tile([BPC, 512], F32, tag=f"smb{h}", name=f"bias_ps{h}")
                   for h in range(2)]
        holder['bias_ps'] = bias_ps
        mean_sb = const.tile([BPC, H], F32)
        for half in range(2):
            nc.tensor.matmul(
                bias_ps[half][:], sel[:],
                asp_sb[:, half * 512 : (half + 1) * 512],
                start=True, stop=True)
            nc.scalar.copy(mean_sb[:, half * 512 : (half + 1) * 512],
                           bias_ps[half][:])

        # transpose mean into [h_in partitions, k, batch]
        meanT = const.tile([128, KC, BPC], F32)
        holder['meanT'] = meanT
        for k in range(KC):
            ps = tp_ps.tile([128, BPC], F32, tag="tp")
            nc.tensor.transpose(
                ps[:], mean_sb[:, k * 128 : (k + 1) * 128], ident[0:BPC, 0:BPC])
            nc.scalar.copy(meanT[:, k, :], ps[:])

    def emit_bias_k(k):
        # bias[b, m] += meanT[k]^T @ W2[k-chunk]: f32 matmuls streamed as
        # each staged W2 chunk lands; cheap startup filler for the PE
        bias_ps = holder['bias_ps']
        w2stg = w2p.tile([128, H], F32, tag="w2stg", bufs=3, name=f"w2stg{k}")
        nc.scalar.dma_start(w2stg[:], W[H + k * 128 : H + (k + 1) * 128, :])
        for half in range(2):
            nc.tensor.matmul(
                bias_ps[half][:], holder['meanT'][:, k, :],
                w2stg[:, half * 512 : (half + 1) * 512],
                start=(k == 0), stop=(k == KC - 1))

    def emit_bias_finish():
        bias_ps = holder['bias_ps']
        bias_sb = const.tile([BPC, H], F32)
        for half in range(2):
            nc.scalar.copy(bias_sb[:, half * 512 : (half + 1) * 512],
                           bias_ps[half][:])

        # transpose bias rows into biasT[m][p, batch] and add b
        biasT = const.tile([128, MC, BPC], F32)
        holder['biasT'] = biasT
        for m in range(MC):
            ps = tp_ps.tile([128, BPC], F32, tag="tp")
            nc.tensor.transpose(
                ps[:], bias_sb[:, m * 128 : (m + 1) * 128], ident[0:BPC, 0:BPC])
            nc.vector.tensor_scalar(
                biasT[:, m, :], ps[:], b_sb[:, m : m + 1], None, op0=ALU.add)

    def emit_group(t, m, mm):
        # E^T m-group: 4 double-row fp8 matmuls (k-pairs)
        # group 0 reads k-pairs in reverse: its first matmul then depends
        # on the final transpose copy, which stops the scheduler from
        # interleaving transposes into the matmul stream
        xT = holder['xT']
        kps = list(reversed(range(KP))) if (m == 0 and t > 0) else list(range(KP))
        for i, kp in enumerate(kps):
            nc.tensor.matmul(
                mm[:],
                w1_sb[:, 2 * kp : 2 * kp + 2, m * 128 : (m + 1) * 128],
                xT[:, 2 * kp : 2 * kp + 2, :],
                start=(i == 0), stop=(i == KP - 1), perf_mode=DR)

    for t in range(NT512):
        bb = t // (NT512 // BPC)
        if t > 0 and t + 2 < NT512:
            load_x(t + 2)

        # cast to fp8, transpose X block -> [h_in part, k, 512 tokens].
        # k-outer so xT chunks complete in the order the matmuls read them.
        xT = xtp.tile([128, KC, 512], F8, tag="xT")
        holder['xT'] = xT
        xbs = []
        for j in range(4):
            n = 4 * t + j
            xb = xbp.tile([128, H], F16, tag="xb", name=f"xb{n}")
            nc.vector.tensor_copy(out=xb[:], in_=x_nat[n][:])
            xbs.append(xb)
        for kp in range(KP):
            ps = tp_ps.tile([128, 1024], F16, tag="tp")
            for kk in range(2):
                k = 2 * kp + kk
                for j in range(4):
                    nc.tensor.transpose(
                        ps[:, kk * 512 + j * 128 : kk * 512 + (j + 1) * 128],
                        xbs[j][:, k * 128 : (k + 1) * 128], identh[:])
            nc.vector.tensor_copy(out=xT[:, 2 * kp : 2 * kp + 2, :], in_=ps[:])

        if t == 0:
            emit_w1_casts()

        # previous block's score-row transposes (its ACT copy has finished
        # during our transpose phase, so the PE does not stall)
        if t - 1 in sc_rows:
            emit_score_transposes(t - 1)

        # E^T = tanh((64*W1)^T @ X^T / 64 + bias); scores += v^T @ E^T
        # (vdot for group m is emitted after matmul group m+1 so the PE
        # never waits on the tanh producing e[m])
        sc_ps = vd_ps.tile([1, 512], F32, tag="vd")
        e_tiles = []

        def tanh_and_vdot(m, mm):
            e = ep.tile([128, 512], F16, tag="e")
            nc.scalar.activation(e[:], mm[:], AF.Tanh,
                                 bias=holder['biasT'][:, m, bb : bb + 1],
                                 scale=1.0 / WSCALE)
            e_tiles.append((m, e))

        if t == 0:
            # block 0: bias-path f32 matmuls interleave with a k-pair-outer
            # sweep over three m-groups, in DMA arrival order, so the PE
            # always has work while W1/W2 stream from HBM; m3-7 run
            # k-pair-inner afterwards (W1 fully resident by then).
            emit_bias_head()
            mms = [mm_ps.tile([128, 512], F32, tag="mm", name=f"mm0_{m}")
                   for m in range(3)]
            for kp in range(KP):
                emit_bias_k(2 * kp)
                emit_bias_k(2 * kp + 1)
                for m in range(3):
                    nc.tensor.matmul(
                        mms[m][:],
                        w1_sb[:, 2 * kp : 2 * kp + 2, m * 128 : (m + 1) * 128],
                        xT[:, 2 * kp : 2 * kp + 2, :],
                        start=(kp == 0), stop=(kp == KP - 1), perf_mode=DR)
            emit_bias_finish()
            load_x(1)
            load_x(2)
            for m in range(3):
                tanh_and_vdot(m, mms[m])
            for m in range(3, MC):
                mm = mm_ps.tile([128, 512], F32, tag="mm")
                emit_group(t, m, mm)
                tanh_and_vdot(m, mm)
        else:
            for m in range(MC):
                mm = mm_ps.tile([128, 512], F32, tag="mm")
                emit_group(t, m, mm)
                tanh_and_vdot(m, mm)

        for pm, pe_t in e_tiles:
            nc.tensor.matmul(
                sc_ps[:], v_sb[:, pm : pm + 1], pe_t[:],
                start=(pm == 0), stop=(pm == MC - 1), skip_group_check=True)
        row = smp.tile([1, 512], F32, tag="scrow", bufs=2, name=f"row{t}")
        nc.scalar.copy(row[:], sc_ps[:])
        sc_rows[t] = row

        # finalize work is emitted AFTER the matmul section: the DVE queue
        # is in-order, and scales queued ahead of the next block's work
        # were stalling the PE at every finalize iteration. Scale+store
        # work is spread 4 tiles per block to avoid engine bursts.
        if pending_fin is not None:
            emit_softmax_and_scales(pending_fin)
            pending_scale = (pending_fin, 0)
            pending_fin = None
        if pending_scale is not None:
            sb_, off = pending_scale
            emit_scales(sb_, range(off, off + 4), [nc.sync, nc.gpsimd],
                        use_act=True)
            pending_scale = (sb_, off + 4) if off + 4 < 16 else None

        if t % (NT512 // BPC) == (NT512 // BPC) - 1:
            pending_fin = bb

    # tail: last block's score transposes + last batch softmax/scales;
    # stores fan out over three queues to saturate HBM write bandwidth
    if pending_scale is not None:
        sb_, off = pending_scale
        emit_scales(sb_, range(off, 16), [nc.sync, nc.gpsimd])
    emit_score_transposes(NT512 - 1)
    emit_softmax_and_scales(pending_fin)
    emit_scales(pending_fin, range(0, 16), [nc.sync, nc.scalar, nc.gpsimd])


_CACHE = {}


def _build():
    if "nc" in _CACHE:
        return _CACHE["nc"]
    nc = bacc.Bacc("TRN2", target_bir_lowering=False, debug=False,
                   num_devices=NCORES)
    tok = nc.dram_tensor("tok", [T, H], F32, kind="ExternalInput").ap()
    asp = nc.dram_tensor("asp", [BPC, A, H], F32, kind="ExternalInput").ap()
    W_ = nc.dram_tensor("W", [2 * H, H], F32, kind="ExternalInput").ap()
    b_ = nc.dram_tensor("b", [H], F32, kind="ExternalInput").ap()
    v_ = nc.dram_tensor("v", [H], F32, kind="ExternalInput").ap()
    outp = nc.dram_tensor("out", [T, H], F32, kind="ExternalOutput").ap()

    with tile.TileContext(nc) as tc:
        with ExitStack() as ctx:
            _emit(ctx, tc, outp, tok, asp, W_, b_, v_)
    nc.compile()
    _CACHE["nc"] = nc
    return nc


def make_in_maps(token_embeddings, aspect_embedding, W, b, v):
    in_maps = []
    for c in range(NCORES):
        in_maps.append({
            "tok": np.ascontiguousarray(
                token_embeddings[BPC * c : BPC * (c + 1)].reshape(T, H)),
            "asp": np.ascontiguousarray(
                aspect_embedding[BPC * c : BPC * (c + 1)]),
            "W": W, "b": b, "v": v,
        })
    return in_maps


def kernel(token_embeddings, aspect_embedding, W, b, v):
    token_embeddings = np.asarray(token_embeddings, dtype=np.float32)
    aspect_embedding = np.asarray(aspect_embedding, dtype=np.float32)
    W = np.asarray(W, dtype=np.float32)
    b = np.asarray(b, dtype=np.float32)
    v = np.asarray(v, dtype=np.float32)

    nc = _build()
    in_maps = make_in_maps(token_embeddings, aspect_embedding, W, b, v)
    res = run_bass_kernel_spmd(nc, in_maps, core_ids=list(range(NCORES)))
    return np.concatenate(
        [res.results[c]["out"].reshape(BPC, S, H) for c in range(NCORES)], axis=0)


# revision 12
# speedup vs baseline: 1.2119x; 1.2054x over previous
"""AspectAttention Trainium2 kernel (8 NeuronCores, batch-parallel, fp8).

out = tok * (1 + softmax_S(tanh(cat(tok, mean_A(asp)) @ W + b) @ v))

Sharding: data-parallel over batch B=16 -> 2 batches per core. Softmax is
per-(batch) row over S, so no cross-core communication is needed.

Per-core math (concat split): E^T = tanh(W1^T @ X^T + biasT), where
biasT = (mean_A(asp) @ W2 + b) is per-batch and precomputed on the host
(a 64KB constant, 0.03% of the FLOPs, replacing a 4MB W2 read per core);
scores = v^T @ E^T; weights = softmax(scores); out = X * (1 + weights).

The heavy matmul runs in fp8e4m3 with perf_mode=DoubleRow (2 k-chunks per
matmul). W1 is pre-scaled by 64 into fp8 to stay out of the subnormal
range; the tanh activation un-scales with scale=1/64. Empirically (exact
problem inputs) this lands at rel err ~1.7e-3 vs the 2e-2 gate.

Schedule notes (from perfetto traces):
- DMA reads sustain only ~230 GB/s aggregate (writes ~410), so the x/W1
  read stream is the spine of the kernel; loads round-robin on all three
  HWDGE queues (sync/gpsimd/scalar) and x tiles prefetch 2 blocks ahead.
- PE-transpose-mode does not count as PE activity for the HAM clock
  governor: a batched 32-transpose burst per block triggers a ~3.4us
  half-clock window. Transposes of block t+1 are therefore interleaved
  4 at a time between the matmul groups of block t.
- Block 0 accumulates k-pair-outer across four PSUM banks so matmuls
  start as W1 chunks arrive instead of waiting for the full 4MB.
"""

from contextlib import ExitStack

import numpy as np

import concourse.bass as bass
import concourse.mybir as mybir
import concourse.tile as tile
from concourse import bacc, bass_isa
from concourse.bass_utils import run_bass_kernel_spmd
from concourse.masks import make_identity

B, S, H, A = 16, 2048, 1024, 8
NCORES = 8
BPC = B // NCORES          # batches per core = 2
T = BPC * S                # tokens per core = 4096
NT = T // 128              # 32 token-128 tiles per core
NT512 = T // 512           # 8 token-512 tiles per core
KC = H // 128              # 8 contraction chunks
KP = KC // 2               # 4 double-row k-pairs
MC = H // 128              # 8 output-dim chunks
WSCALE = 64.0              # W1 fp8 pre-scale

F32 = mybir.dt.float32
F16 = mybir.dt.float16
F8 = mybir.dt.float8e4
ALU = mybir.AluOpType
AF = mybir.ActivationFunctionType
AX = mybir.AxisListType
DR = mybir.MatmulPerfMode.DoubleRow


def _emit(ctx: ExitStack, tc: "tile.TileContext", out, tok, W, biasT_in, vvec):
    nc = tc.nc

    const = ctx.enter_context(tc.tile_pool(name="const", bufs=1))
    xres = ctx.enter_context(tc.tile_pool(name="xres", bufs=28))
    xtp = ctx.enter_context(tc.tile_pool(name="xtp", bufs=2))
    xbp = ctx.enter_context(tc.tile_pool(name="xbp", bufs=8))
    ep = ctx.enter_context(tc.tile_pool(name="ep", bufs=9))
    wsp = ctx.enter_context(tc.tile_pool(name="wsp", bufs=4))
    smp = ctx.enter_context(tc.tile_pool(name="smp", bufs=1))

    # PSUM is 8 banks, bank-granular: mm 4 + tp 3 + vd 1
    mm_ps = ctx.enter_context(tc.tile_pool(name="mm_ps", bufs=4, space="PSUM"))
    tp_ps = ctx.enter_context(tc.tile_pool(name="tp_ps", bufs=3, space="PSUM"))
    vd_ps = ctx.enter_context(tc.tile_pool(name="vd_ps", bufs=1, space="PSUM"))

    s_sb = const.tile([128, NT], F32)   # per-token-tile (1 + weight) scales

    # HAM warmup: dummy matmuls keep the PE busy from t=0 while the first
    # DMAs land, so the clock is at 8/8 when real work starts. The result
    # is copied into s_sb[:, 0:1], which is rewritten by the softmax
    # scales before any consumer reads it.
    warm = const.tile([128, 512], F8)
    nc.vector.memset(warm[:], 0.0)
    wps = mm_ps.tile([128, 512], F32, tag="mm", name="warm_ps")
    for r in range(26):
        nc.tensor.matmul(wps[:], warm[:, 0:128], warm[:],
                         start=True, stop=True, skip_group_check=True)
    nc.vector.tensor_copy(out=s_sb[:, 0:1], in_=wps[:, 0:1])

    # ---- constants / small inputs -------------------------------------
    ident = const.tile([128, 128], F32)
    make_identity(nc, ident[:])
    identh = const.tile([128, 128], F16)
    make_identity(nc, identh[:])

    v_sb = const.tile([128, MC], F16)            # v[m*128+p] at [p, m]
    v_stg = const.tile([128, MC], F32)
    nc.scalar.dma_start(v_stg[:], vvec.rearrange("(m p) -> p m", p=128))
    nc.vector.tensor_copy(out=v_sb[:], in_=v_stg[:])

    biasT = const.tile([128, MC, BPC], F32)      # bias[b, m*128+p] at [p, m, b]
    nc.scalar.dma_start(biasT[:], biasT_in.rearrange("(m p) b -> p m b", p=128))

    # ---- main pipeline ------------------------------------------------
    tok_t = tok.rearrange("(n p) h -> n p h", p=128)
    out_t = out.rearrange("(n p) h -> n p h", p=128)

    x_nat = {}          # n -> resident [128, H] f32 tile
    sc_rows = {}        # t -> [1, 512] score row awaiting transpose
    sT = {}             # bb -> [128, 16] transposed scores
    for bb in range(BPC):
        sT[bb] = smp.tile([128, 16], F32, tag=f"sT{bb}", name=f"sT{bb}")

    LOAD_Q = [nc.sync, nc.gpsimd, nc.scalar]

    def load_x(t):
        for j in range(4):
            n = 4 * t + j
            xt_ = xres.tile([128, H], F32, tag="x", name=f"x{n}")
            x_nat[n] = xt_
            LOAD_Q[n % 3].dma_start(xt_[:], tok_t[n])

    def emit_score_transpose(t, jj):
        bb = t // (NT512 // BPC)
        row = sc_rows[t]
        col = 4 * (t % (NT512 // BPC)) + jj
        ps = tp_ps.tile([128, 1], F32, tag="tp")
        nc.tensor.transpose(
            ps[:], row[0:1, jj * 128 : (jj + 1) * 128], ident[0:1, 0:1])
        nc.vector.tensor_copy(out=sT[bb][:, col : col + 1], in_=ps[:])

    def emit_softmax(bb):
        # softmax over the transposed [128, 16] score block, then the
        # per-token scale s = 1 + exp(x - max)/sum
        stile = sT[bb]
        pmax = smp.tile([128, 1], F32, tag="pmax", name=f"pmax{bb}")
        nc.vector.tensor_reduce(pmax[:], stile[:], axis=AX.X, op=ALU.max)
        gmax = smp.tile([128, 1], F32, tag="gmax", name=f"gmax{bb}")
        nc.gpsimd.partition_all_reduce(
            gmax[:], pmax[:], channels=128, reduce_op=bass_isa.ReduceOp.max)
        negmax = smp.tile([128, 1], F32, tag="negmax", name=f"negmax{bb}")
        nc.vector.tensor_scalar(negmax[:], gmax[:], -1.0, None, op0=ALU.mult)
        acc = smp.tile([128, 1], F32, tag="acc", name=f"acc{bb}")
        sl = s_sb[:, bb * 16 : (bb + 1) * 16]
        nc.scalar.activation(sl, stile[:], AF.Exp, bias=negmax[:],
                             accum_out=acc[:])
        gsum = smp.tile([128, 1], F32, tag="gsum", name=f"gsum{bb}")
        nc.gpsimd.partition_all_reduce(
            gsum[:], acc[:], channels=128, reduce_op=bass_isa.ReduceOp.add)
        rc = smp.tile([128, 1], F32, tag="rc", name=f"rc{bb}")
        nc.vector.reciprocal(rc[:], gsum[:])
        nc.vector.tensor_scalar(sl, sl, rc[:], 1.0, op0=ALU.mult, op1=ALU.add)

    def emit_scales(bb, js, queues, use_act=False):
        for i, j in enumerate(js):
            n = bb * (NT // BPC) + j
            if use_act and i % 2 == 1:
                nc.scalar.mul(x_nat[n][:], x_nat[n][:], s_sb[:, n : n + 1])
            else:
                nc.vector.tensor_scalar(
                    x_nat[n][:], x_nat[n][:], s_sb[:, n : n + 1], None,
                    op0=ALU.mult)
            queues[i % len(queues)].dma_start(out_t[n], x_nat[n][:])

    def emit_casts(t):
        xbs = []
        for j in range(4):
            n = 4 * t + j
            xb = xbp.tile([128, H], F16, tag="xb", name=f"xb{n}")
            nc.vector.tensor_copy(out=xb[:], in_=x_nat[n][:])
            xbs.append(xb)
        return xbs

    def emit_transpose_pair(xbs, xT, kp):
        # 8 PE transposes (2 k-chunks x 4 token tiles) + one DVE copy
        ps = tp_ps.tile([128, 1024], F16, tag="tp")
        for kk in range(2):
            k = 2 * kp + kk
            for j in range(4):
                nc.tensor.transpose(
                    ps[:, kk * 512 + j * 128 : kk * 512 + (j + 1) * 128],
                    xbs[j][:, k * 128 : (k + 1) * 128], identh[:])
        nc.vector.tensor_copy(out=xT[:, 2 * kp : 2 * kp + 2, :], in_=ps[:])

    # ---- startup DMAs: x0-3, then W1 (4MB) split over all 3 queues ----
    load_x(0)
    w1_sb = const.tile([128, KC, H], F8)         # 64*W1[k*128+p, m] at [p, k, m]
    w1_src = W[0:H, :].rearrange("(k p) m -> p k m", p=128)
    w1stg = {}
    for k in range(KC):
        stg = wsp.tile([128, H], F32, tag="w1stg", bufs=4, name=f"w1stg{k}")
        LOAD_Q[k % 3].dma_start(stg[:], w1_src[:, k])
        w1stg[k] = stg
    load_x(1)
    load_x(2)

    def emit_w1_casts(ks):
        for k in ks:
            nc.vector.tensor_scalar(
                w1_sb[:, k, :], w1stg[k][:], WSCALE, None, op0=ALU.mult)

    # block 0's fp16 casts + transposes run while W1 streams in
    xT_cur = xtp.tile([128, KC, 512], F8, tag="xT", name="xT0")
    xbs0 = emit_casts(0)
    for kp in range(KP):
        emit_transpose_pair(xbs0, xT_cur, kp)

    pending_fin = None
    pending_scale = None

    for t in range(NT512):
        bb = t // (NT512 // BPC)
        if t >= 1 and t + 2 < NT512:
            load_x(t + 2)

        # next block's casts (DVE) + its xT tile; transposes interleave
        # into this block's matmul stream below
        xT_next = None
        xbs_next = None
        if t + 1 < NT512:
            if t == 0:
                emit_w1_casts(range(0, 4))
            xbs_next = emit_casts(t + 1)
            if t == 0:
                emit_w1_casts(range(4, 8))
            xT_next = xtp.tile([128, KC, 512], F8, tag="xT", name=f"xT{t+1}")

        sc_ps = vd_ps.tile([1, 512], F32, tag="vd")
        e_tiles = []

        def tanh_group(m, mm):
            e = ep.tile([128, 512], F16, tag="e")
            nc.scalar.activation(e[:], mm[:], AF.Tanh,
                                 bias=biasT[:, m, bb : bb + 1],
                                 scale=1.0 / WSCALE)
            e_tiles.append((m, e))

        def after_group(m):
            # interleave next-block transposes and prev-block score
            # transposes between matmul groups: PE-transpose-mode does not
            # count as HAM activity, so bursts of them downclock the core
            if m % 2 == 1 and xbs_next is not None:
                emit_transpose_pair(xbs_next, xT_next, (m - 1) // 2)
            if m % 2 == 0 and t - 1 in sc_rows:
                emit_score_transpose(t - 1, m // 2)

        if t == 0:
            # k-pair-outer over four m-groups: matmuls start as W1 chunk
            # pairs arrive from HBM; m4-7 run k-pair-inner afterwards.
            mms = [mm_ps.tile([128, 512], F32, tag="mm", name=f"mm0_{m}")
                   for m in range(4)]
            for kp in range(KP):
                for m in range(4):
                    nc.tensor.matmul(
                        mms[m][:],
                        w1_sb[:, 2 * kp : 2 * kp + 2, m * 128 : (m + 1) * 128],
                        xT_cur[:, 2 * kp : 2 * kp + 2, :],
                        start=(kp == 0), stop=(kp == KP - 1), perf_mode=DR)
            for m in range(4):
                tanh_group(m, mms[m])
                after_group(m)
            for m in range(4, MC):
                mm = mm_ps.tile([128, 512], F32, tag="mm")
                for i, kp in enumerate(range(KP)):
                    nc.tensor.matmul(
                        mm[:],
                        w1_sb[:, 2 * kp : 2 * kp + 2, m * 128 : (m + 1) * 128],
                        xT_cur[:, 2 * kp : 2 * kp + 2, :],
                        start=(i == 0), stop=(i == KP - 1), perf_mode=DR)
                tanh_group(m, mm)
                after_group(m)
        else:
            for m in range(MC):
                mm = mm_ps.tile([128, 512], F32, tag="mm")
                # group 0 reads k-pairs in reverse: its first matmul then
                # depends on the final transpose copy, which keeps the
                # scheduler from racing ahead of the transpose stream
                kps = list(reversed(range(KP))) if m == 0 else list(range(KP))
                for i, kp in enumerate(kps):
                    nc.tensor.matmul(
                        mm[:],
                        w1_sb[:, 2 * kp : 2 * kp + 2, m * 128 : (m + 1) * 128],
                        xT_cur[:, 2 * kp : 2 * kp + 2, :],
                        start=(i == 0), stop=(i == KP - 1), perf_mode=DR)
                tanh_group(m, mm)
                after_group(m)

        # batched v-dots: fp16 matmuls count as PE activity and run at
        # full rate back-to-back; one fp8<->fp16 mode switch per block
        for pm, pe_t in e_tiles:
            nc.tensor.matmul(
                sc_ps[:], v_sb[:, pm : pm + 1], pe_t[:],
                start=(pm == 0), stop=(pm == MC - 1), skip_group_check=True)
        row = smp.tile([1, 512], F32, tag="scrow", bufs=2, name=f"row{t}")
        nc.scalar.copy(row[:], sc_ps[:])
        if t - 1 in sc_rows:
            del sc_rows[t - 1]
        sc_rows[t] = row
        xT_cur = xT_next

        # finalize: softmax once a batch's scores are complete, then
        # scale+store spread 4 tiles per block to avoid engine bursts
        if pending_fin is not None:
            emit_softmax(pending_fin)
            pending_scale = (pending_fin, 0)
            pending_fin = None
        if pending_scale is not None:
            sb_, off = pending_scale
            emit_scales(sb_, range(off, off + 4), [nc.sync, nc.gpsimd],
                        use_act=True)
            pending_scale = (sb_, off + 4) if off + 4 < 16 else None

        if t % (NT512 // BPC) == (NT512 // BPC) - 1:
            pending_fin = bb

    # tail: remaining score transposes + last batch softmax/scales;
    # stores fan out over all three queues
    if pending_scale is not None:
        sb_, off = pending_scale
        emit_scales(sb_, range(off, 16), [nc.sync, nc.gpsimd])
    tlast = NT512 - 1
    for jj in range(4):
        emit_score_transpose(tlast, jj)
    emit_softmax(pending_fin)
    emit_scales(pending_fin, range(0, 16), [nc.sync, nc.scalar, nc.gpsimd])


_CACHE = {}


def _build():
    if "nc" in _CACHE:
        return _CACHE["nc"]
    nc = bacc.Bacc("TRN2", target_bir_lowering=False, debug=False,
                   num_devices=NCORES)
    tok = nc.dram_tensor("tok", [T, H], F32, kind="ExternalInput").ap()
    W_ = nc.dram_tensor("W", [2 * H, H], F32, kind="ExternalInput").ap()
    bT = nc.dram_tensor("biasT", [H, BPC], F32, kind="ExternalInput").ap()
    v_ = nc.dram_tensor("v", [H], F32, kind="ExternalInput").ap()
    outp = nc.dram_tensor("out", [T, H], F32, kind="ExternalOutput").ap()

    with tile.TileContext(nc) as tc:
        with ExitStack() as ctx:
            _emit(ctx, tc, outp, tok, W_, bT, v_)
    nc.compile()
    _CACHE["nc"] = nc
    return nc


def host_bias(aspect_embedding, W, b):
    """bias[b, h] = mean_A(asp)[b] @ W2 + b  (64KB constant, on host)."""
    am = aspect_embedding.astype(np.float64).mean(axis=1)      # [B, H]
    return (am @ W.astype(np.float64)[H:] + b.astype(np.float64)).astype(
        np.float32)                                            # [B, H]


def make_in_maps(token_embeddings, aspect_embedding, W, b, v):
    bias = host_bias(aspect_embedding, W, b)
    in_maps = []
    for c in range(NCORES):
        in_maps.append({
            "tok": np.ascontiguousarray(
                token_embeddings[BPC * c : BPC * (c + 1)].reshape(T, H)),
            "biasT": np.ascontiguousarray(
                bias[BPC * c : BPC * (c + 1)].T),              # [H, BPC]
            "W": W, "v": v,
        })
    return in_maps


def kernel(token_embeddings, aspect_embedding, W, b, v):
    token_embeddings = np.asarray(token_embeddings, dtype=np.float32)
    aspect_embedding = np.asarray(aspect_embedding, dtype=np.float32)
    W = np.asarray(W, dtype=np.float32)
    b = np.asarray(b, dtype=np.float32)
    v = np.asarray(v, dtype=np.float32)

    nc = _build()
    in_maps = make_in_maps(token_embeddings, aspect_embedding, W, b, v)
    res = run_bass_kernel_spmd(nc, in_maps, core_ids=list(range(NCORES)))
    return np.concatenate(
        [res.results[c]["out"].reshape(BPC, S, H) for c in range(NCORES)], axis=0)


# revision 14
# speedup vs baseline: 1.3066x; 1.0781x over previous
"""AspectAttention Trainium2 kernel (8 NeuronCores, batch-parallel, fp8).

out = tok * (1 + softmax_S(tanh(cat(tok, mean_A(asp)) @ W + b) @ v))

Sharding: data-parallel over batch B=16 -> 2 batches per core. Softmax is
per-(batch) row over S, so no cross-core communication is needed.

Per-core math (concat split): E^T = tanh(W1^T @ X^T + biasT), where
biasT = (mean_A(asp) @ W2 + b) is per-batch and precomputed on the host
(a 64KB constant, 0.03% of the FLOPs, replacing a 4MB W2 read per core);
scores = v^T @ E^T; weights = softmax(scores); out = X * (1 + weights).

The heavy matmul runs in fp8e4m3 with perf_mode=DoubleRow (2 k-chunks per
matmul). W1 is pre-scaled by 64 into fp8 to stay out of the subnormal
range; the tanh activation un-scales with scale=1/64. Empirically (exact
problem inputs) this lands at rel err ~1.7e-3 vs the 2e-2 gate.

Schedule notes (from perfetto traces):
- DMA reads sustain only ~230 GB/s aggregate (writes ~410), so the x/W1
  read stream is the spine of the kernel; loads round-robin on all three
  HWDGE queues (sync/gpsimd/scalar) and x tiles prefetch 2 blocks ahead.
- PE-transpose-mode does not count as PE activity for the HAM clock
  governor: a batched 32-transpose burst per block triggers a ~3.4us
  half-clock window. Transposes of block t+1 are therefore interleaved
  4 at a time between the matmul groups of block t.
- Block 0 accumulates k-pair-outer across four PSUM banks so matmuls
  start as W1 chunks arrive instead of waiting for the full 4MB.
"""

from contextlib import ExitStack

import numpy as np

import concourse.bass as bass
import concourse.mybir as mybir
import concourse.tile as tile
from concourse import bacc, bass_isa
from concourse.bass_utils import run_bass_kernel_spmd
from concourse.masks import make_identity

B, S, H, A = 16, 2048, 1024, 8
NCORES = 8
BPC = B // NCORES          # batches per core = 2
T = BPC * S                # tokens per core = 4096
NT = T // 128              # 32 token-128 tiles per core
NT512 = T // 512           # 8 token-512 tiles per core
KC = H // 128              # 8 contraction chunks
KP = KC // 2               # 4 double-row k-pairs
MC = H // 128              # 8 output-dim chunks
WSCALE = 64.0              # W1 fp8 pre-scale

F32 = mybir.dt.float32
F16 = mybir.dt.float16
F8 = mybir.dt.float8e4
ALU = mybir.AluOpType
AF = mybir.ActivationFunctionType
AX = mybir.AxisListType
DR = mybir.MatmulPerfMode.DoubleRow


def _emit(ctx: ExitStack, tc: "tile.TileContext", out, tok, W, biasT_in, vvec):
    nc = tc.nc

    const = ctx.enter_context(tc.tile_pool(name="const", bufs=1))
    xres = ctx.enter_context(tc.tile_pool(name="xres", bufs=28))
    xtp = ctx.enter_context(tc.tile_pool(name="xtp", bufs=2))
    xbp = ctx.enter_context(tc.tile_pool(name="xbp", bufs=8))
    ep = ctx.enter_context(tc.tile_pool(name="ep", bufs=9))
    wsp = ctx.enter_context(tc.tile_pool(name="wsp", bufs=4))
    smp = ctx.enter_context(tc.tile_pool(name="smp", bufs=1))

    # PSUM is 8 banks, bank-granular: mm 4 + tp 3 + vd 1
    mm_ps = ctx.enter_context(tc.tile_pool(name="mm_ps", bufs=4, space="PSUM"))
    tp_ps = ctx.enter_context(tc.tile_pool(name="tp_ps", bufs=3, space="PSUM"))
    vd_ps = ctx.enter_context(tc.tile_pool(name="vd_ps", bufs=1, space="PSUM"))

    s_sb = const.tile([128, NT], F32)   # per-token-tile (1 + weight) scales

    # HAM warmup: dummy matmuls keep the PE busy from t=0 while the first
    # DMAs land, so the clock is at 8/8 when real work starts. The result
    # is copied into s_sb[:, 0:1], which is rewritten by the softmax
    # scales before any consumer reads it.
    warm = const.tile([128, 512], F8)
    nc.vector.memset(warm[:], 0.0)
    wps = None
    for r in range(16):
        wps = mm_ps.tile([128, 512], F32, tag="mm", name=f"warm_ps{r % 4}")
        nc.tensor.matmul(wps[:], warm[:, 0:128], warm[:],
                         start=True, stop=True, skip_group_check=True)
    nc.vector.tensor_copy(out=s_sb[:, 0:1], in_=wps[:, 0:1])

    # ---- constants / small inputs -------------------------------------
    ident = const.tile([128, 128], F32)
    make_identity(nc, ident[:])
    ident8 = const.tile([128, 128], F8)
    make_identity(nc, ident8[:])

    v_sb = const.tile([128, MC], F16)            # v[m*128+p] at [p, m]
    v_stg = const.tile([128, MC], F32)
    nc.scalar.dma_start(v_stg[:], vvec.rearrange("(m p) -> p m", p=128))
    nc.vector.tensor_copy(out=v_sb[:], in_=v_stg[:])

    biasT = const.tile([128, MC, BPC], F32)      # bias[b, m*128+p] at [p, m, b]
    nc.scalar.dma_start(biasT[:], biasT_in.rearrange("(m p) b -> p m b", p=128))

    # ---- main pipeline ------------------------------------------------
    tok_t = tok.rearrange("(n p) h -> n p h", p=128)
    out_t = out.rearrange("(n p) h -> n p h", p=128)

    x_nat = {}          # n -> resident [128, H] f32 tile
    sc_rows = {}        # t -> [1, 512] score row awaiting transpose
    sT = {}             # bb -> [128, 16] transposed scores
    for bb in range(BPC):
        sT[bb] = smp.tile([128, 16], F32, tag=f"sT{bb}", name=f"sT{bb}")

    def load_x(t):
        for j in range(4):
            n = 4 * t + j
            xt_ = xres.tile([128, H], F32, tag="x", name=f"x{n}")
            x_nat[n] = xt_
            if n >= 16 and n % 4 == 3:
                eng = nc.scalar
            else:
                eng = nc.sync if n % 2 == 0 else nc.gpsimd
            eng.dma_start(xt_[:], tok_t[n])

    def emit_score_transpose(t, jj):
        bb = t // (NT512 // BPC)
        row = sc_rows[t]
        col = 4 * (t % (NT512 // BPC)) + jj
        ps = tp_ps.tile([128, 1], F32, tag="tp")
        nc.tensor.transpose(
            ps[:], row[0:1, jj * 128 : (jj + 1) * 128], ident[0:1, 0:1])
        nc.vector.tensor_copy(out=sT[bb][:, col : col + 1], in_=ps[:])

    def emit_softmax(bb):
        # softmax over the transposed [128, 16] score block, then the
        # per-token scale s = 1 + exp(x - max)/sum
        stile = sT[bb]
        pmax = smp.tile([128, 1], F32, tag="pmax", name=f"pmax{bb}")
        nc.vector.tensor_reduce(pmax[:], stile[:], axis=AX.X, op=ALU.max)
        gmax = smp.tile([128, 1], F32, tag="gmax", name=f"gmax{bb}")
        nc.gpsimd.partition_all_reduce(
            gmax[:], pmax[:], channels=128, reduce_op=bass_isa.ReduceOp.max)
        negmax = smp.tile([128, 1], F32, tag="negmax", name=f"negmax{bb}")
        nc.vector.tensor_scalar(negmax[:], gmax[:], -1.0, None, op0=ALU.mult)
        acc = smp.tile([128, 1], F32, tag="acc", name=f"acc{bb}")
        sl = s_sb[:, bb * 16 : (bb + 1) * 16]
        nc.scalar.activation(sl, stile[:], AF.Exp, bias=negmax[:],
                             accum_out=acc[:])
        gsum = smp.tile([128, 1], F32, tag="gsum", name=f"gsum{bb}")
        nc.gpsimd.partition_all_reduce(
            gsum[:], acc[:], channels=128, reduce_op=bass_isa.ReduceOp.add)
        rc = smp.tile([128, 1], F32, tag="rc", name=f"rc{bb}")
        nc.vector.reciprocal(rc[:], gsum[:])
        nc.vector.tensor_scalar(sl, sl, rc[:], 1.0, op0=ALU.mult, op1=ALU.add)

    def emit_scales(bb, js, queues, use_act=False):
        for i, j in enumerate(js):
            n = bb * (NT // BPC) + j
            if use_act and i % 2 == 1:
                nc.scalar.mul(x_nat[n][:], x_nat[n][:], s_sb[:, n : n + 1])
            else:
                nc.vector.tensor_scalar(
                    x_nat[n][:], x_nat[n][:], s_sb[:, n : n + 1], None,
                    op0=ALU.mult)
            queues[i % len(queues)].dma_start(out_t[n], x_nat[n][:])

    def emit_casts(t):
        xbs = []
        for j in range(4):
            n = 4 * t + j
            xb = xbp.tile([128, H], F8, tag="xb", name=f"xb{n}")
            nc.vector.tensor_copy(out=xb[:], in_=x_nat[n][:])
            xbs.append(xb)
        return xbs

    def emit_transpose_pair(xbs, xT, kp):
        # 8 PE transposes (2 k-chunks x 4 token tiles) + one DVE copy.
        # fp8 transpose-mode writes PSUM with element step 2 (HW rule).
        ps = tp_ps.tile([128, 1024, 2], F8, tag="tp")
        for kk in range(2):
            k = 2 * kp + kk
            for j in range(4):
                nc.tensor.transpose(
                    ps[:, kk * 512 + j * 128 : kk * 512 + (j + 1) * 128, 0],
                    xbs[j][:, k * 128 : (k + 1) * 128], ident8[:])
        nc.vector.tensor_copy(out=xT[:, 2 * kp : 2 * kp + 2, :],
                              in_=ps[:, :, 0])

    # ---- startup DMAs: x0-3, then W1 (4MB) split over all 3 queues ----
    load_x(0)
    w1_sb = const.tile([128, KC, H], F8)         # 64*W1[k*128+p, m] at [p, k, m]
    w1_src = W[0:H, :].rearrange("(k p) m -> p k m", p=128)
    w1stg = {}
    for k in range(KC):
        stg = wsp.tile([128, H], F32, tag="w1stg", bufs=4, name=f"w1stg{k}")
        (nc.sync if k % 2 == 0 else nc.gpsimd).dma_start(stg[:], w1_src[:, k])
        w1stg[k] = stg
    load_x(1)
    load_x(2)

    def emit_w1_casts(ks):
        for k in ks:
            nc.vector.tensor_scalar(
                w1_sb[:, k, :], w1stg[k][:], WSCALE, None, op0=ALU.mult)

    # block 0's fp16 casts + transposes run while W1 streams in
    xT_cur = xtp.tile([128, KC, 512], F8, tag="xT", name="xT0")
    xbs0 = emit_casts(0)
    for kp in range(KP):
        emit_transpose_pair(xbs0, xT_cur, kp)

    pending_fin = None
    pending_scale = None

    for t in range(NT512):
        bb = t // (NT512 // BPC)
        if t >= 1 and t + 2 < NT512:
            load_x(t + 2)

        # next block's casts (DVE) + its xT tile; transposes interleave
        # into this block's matmul stream below
        xT_next = None
        xbs_next = None
        if t + 1 < NT512:
            if t == 0:
                emit_w1_casts(range(0, 4))
            xbs_next = emit_casts(t + 1)
            if t == 0:
                emit_w1_casts(range(4, 8))
            xT_next = xtp.tile([128, KC, 512], F8, tag="xT", name=f"xT{t+1}")

        sc_ps = vd_ps.tile([1, 512], F32, tag="vd")
        e_tiles = []

        def tanh_group(m, mm):
            e = ep.tile([128, 512], F16, tag="e")
            nc.scalar.activation(e[:], mm[:], AF.Tanh,
                                 bias=biasT[:, m, bb : bb + 1],
                                 scale=1.0 / WSCALE)
            e_tiles.append((m, e))

        def after_group(m):
            # interleave next-block transposes and prev-block score
            # transposes between matmul groups: PE-transpose-mode does not
            # count as HAM activity, so bursts of them downclock the core
            if m % 2 == 1 and xbs_next is not None:
                emit_transpose_pair(xbs_next, xT_next, (m - 1) // 2)
            if m % 2 == 0 and t - 1 in sc_rows:
                emit_score_transpose(t - 1, m // 2)

        if t == 0:
            # k-pair-outer over four m-groups: matmuls start as W1 chunk
            # pairs arrive from HBM; m4-7 run k-pair-inner afterwards.
            mms = [mm_ps.tile([128, 512], F32, tag="mm", name=f"mm0_{m}")
                   for m in range(4)]
            fill = vd_ps.tile([1, 512], F32, tag="vd", name="fill")
            for kp in range(KP):
                for m in range(4):
                    nc.tensor.matmul(
                        mms[m][:],
                        w1_sb[:, 2 * kp : 2 * kp + 2, m * 128 : (m + 1) * 128],
                        xT_cur[:, 2 * kp : 2 * kp + 2, :],
                        start=(kp == 0), stop=(kp == KP - 1), perf_mode=DR)
                if kp < KP - 1:
                    for _ in range(4):
                        nc.tensor.matmul(
                            fill[:], warm[:, 0:1], warm[:, 0:512],
                            start=True, stop=True, skip_group_check=True)
            for m in range(4):
                tanh_group(m, mms[m])
                after_group(m)
            for m in range(4, MC):
                mm = mm_ps.tile([128, 512], F32, tag="mm")
                for i, kp in enumerate(range(KP)):
                    nc.tensor.matmul(
                        mm[:],
                        w1_sb[:, 2 * kp : 2 * kp + 2, m * 128 : (m + 1) * 128],
                        xT_cur[:, 2 * kp : 2 * kp + 2, :],
                        start=(i == 0), stop=(i == KP - 1), perf_mode=DR)
                tanh_group(m, mm)
                after_group(m)
        else:
            for m in range(MC):
                mm = mm_ps.tile([128, 512], F32, tag="mm")
                # group 0 reads k-pairs in reverse: its first matmul then
                # depends on the final transpose copy, which keeps the
                # scheduler from racing ahead of the transpose stream
                kps = list(reversed(range(KP))) if m == 0 else list(range(KP))
                for i, kp in enumerate(kps):
                    nc.tensor.matmul(
                        mm[:],
                        w1_sb[:, 2 * kp : 2 * kp + 2, m * 128 : (m + 1) * 128],
                        xT_cur[:, 2 * kp : 2 * kp + 2, :],
                        start=(i == 0), stop=(i == KP - 1), perf_mode=DR)
                tanh_group(m, mm)
                after_group(m)

        # batched v-dots: fp16 matmuls count as PE activity and run at
        # full rate back-to-back; one fp8<->fp16 mode switch per block
        for pm, pe_t in e_tiles:
            nc.tensor.matmul(
                sc_ps[:], v_sb[:, pm : pm + 1], pe_t[:],
                start=(pm == 0), stop=(pm == MC - 1), skip_group_check=True)
        row = smp.tile([1, 512], F32, tag="scrow", bufs=2, name=f"row{t}")
        nc.scalar.copy(row[:], sc_ps[:])
        if t - 1 in sc_rows:
            del sc_rows[t - 1]
        sc_rows[t] = row
        xT_cur = xT_next

        # finalize: softmax once a batch's scores are complete, then
        # scale+store spread 4 tiles per block to avoid engine bursts
        if pending_fin is not None:
            emit_softmax(pending_fin)
            pending_scale = (pending_fin, 0)
            pending_fin = None
        if pending_scale is not None:
            sb_, off = pending_scale
            emit_scales(sb_, range(off, off + 4), [nc.sync, nc.gpsimd],
                        use_act=True)
            pending_scale = (sb_, off + 4) if off + 4 < 16 else None

        if t % (NT512 // BPC) == (NT512 // BPC) - 1:
            pending_fin = bb

    # tail: remaining score transposes + last batch softmax/scales;
    # stores fan out over all three queues
    if pending_scale is not None:
        sb_, off = pending_scale
        emit_scales(sb_, range(off, 16), [nc.sync, nc.gpsimd])
    tlast = NT512 - 1
    for jj in range(4):
        emit_score_transpose(tlast, jj)
    emit_softmax(pending_fin)
    emit_scales(pending_fin, range(0, 16), [nc.sync, nc.scalar, nc.gpsimd])


_CACHE = {}


def _build():
    if "nc" in _CACHE:
        return _CACHE["nc"]
    nc = bacc.Bacc("TRN2", target_bir_lowering=False, debug=False,
                   num_devices=NCORES)
    tok = nc.dram_tensor("tok", [T, H], F32, kind="ExternalInput").ap()
    W_ = nc.dram_tensor("W", [2 * H, H], F32, kind="ExternalInput").ap()
    bT = nc.dram_tensor("biasT", [H, BPC], F32, kind="ExternalInput").ap()
    v_ = nc.dram_tensor("v", [H], F32, kind="ExternalInput").ap()
    outp = nc.dram_tensor("out", [T, H], F32, kind="ExternalOutput").ap()

    with tile.TileContext(nc) as tc:
        with ExitStack() as ctx:
            _emit(ctx, tc, outp, tok, W_, bT, v_)
    nc.compile()
    _CACHE["nc"] = nc
    return nc


def host_bias(aspect_embedding, W, b):
    """bias[b, h] = mean_A(asp)[b] @ W2 + b  (64KB constant, on host)."""
    am = aspect_embedding.astype(np.float64).mean(axis=1)      # [B, H]
    return (am @ W.astype(np.float64)[H:] + b.astype(np.float64)).astype(
        np.float32)                                            # [B, H]


def make_in_maps(token_embeddings, aspect_embedding, W, b, v):
    bias = host_bias(aspect_embedding, W, b)
    in_maps = []
    for c in range(NCORES):
        in_maps.append({
            "tok": np.ascontiguousarray(
                token_embeddings[BPC * c : BPC * (c + 1)].reshape(T, H)),
            "biasT": np.ascontiguousarray(
                bias[BPC * c : BPC * (c + 1)].T),              # [H, BPC]
            "W": W, "v": v,
        })
    return in_maps


def kernel(token_embeddings, aspect_embedding, W, b, v):
    token_embeddings = np.asarray(token_embeddings, dtype=np.float32)
    aspect_embedding = np.asarray(aspect_embedding, dtype=np.float32)
    W = np.asarray(W, dtype=np.float32)
    b = np.asarray(b, dtype=np.float32)
    v = np.asarray(v, dtype=np.float32)

    nc = _build()
    in_maps = make_in_maps(token_embeddings, aspect_embedding, W, b, v)
    res = run_bass_kernel_spmd(nc, in_maps, core_ids=list(range(NCORES)))
    return np.concatenate(
        [res.results[c]["out"].reshape(BPC, S, H) for c in range(NCORES)], axis=0)


# revision 15
# speedup vs baseline: 1.3211x; 1.0111x over previous
"""AspectAttention Trainium2 kernel (8 NeuronCores, batch-parallel, fp8).

out = tok * (1 + softmax_S(tanh(cat(tok, mean_A(asp)) @ W + b) @ v))

Sharding: data-parallel over batch B=16 -> 2 batches per core. Softmax is
per-(batch) row over S, so no cross-core communication is needed.

Per-core math (concat split): E^T = tanh(W1^T @ X^T + biasT), where
biasT = (mean_A(asp) @ W2 + b) is per-batch and precomputed on the host
(a 64KB constant, 0.03% of the FLOPs, replacing a 4MB W2 read per core);
scores = v^T @ E^T; weights = softmax(scores); out = X * (1 + weights).

The heavy matmul runs in fp8e4m3 with perf_mode=DoubleRow (2 k-chunks per
matmul). W1 is pre-scaled by 64 into fp8 to stay out of the subnormal
range; the tanh activation un-scales with scale=1/64. Empirically (exact
problem inputs) this lands at rel err ~1.7e-3 vs the 2e-2 gate.

Schedule notes (from perfetto traces):
- DMA reads sustain only ~230 GB/s aggregate (writes ~410), so the x/W1
  read stream is the spine of the kernel; loads round-robin on all three
  HWDGE queues (sync/gpsimd/scalar) and x tiles prefetch 2 blocks ahead.
- PE-transpose-mode does not count as PE activity for the HAM clock
  governor: a batched 32-transpose burst per block triggers a ~3.4us
  half-clock window. Transposes of block t+1 are therefore interleaved
  4 at a time between the matmul groups of block t.
- Block 0 accumulates k-pair-outer across four PSUM banks so matmuls
  start as W1 chunks arrive instead of waiting for the full 4MB.
"""

from contextlib import ExitStack

import numpy as np

import concourse.bass as bass
import concourse.mybir as mybir
import concourse.tile as tile
from concourse import bacc, bass_isa
from concourse.bass_utils import run_bass_kernel_spmd
from concourse.masks import make_identity

B, S, H, A = 16, 2048, 1024, 8
NCORES = 8
BPC = B // NCORES          # batches per core = 2
T = BPC * S                # tokens per core = 4096
NT = T // 128              # 32 token-128 tiles per core
NT512 = T // 512           # 8 token-512 tiles per core
KC = H // 128              # 8 contraction chunks
KP = KC // 2               # 4 double-row k-pairs
MC = H // 128              # 8 output-dim chunks
WSCALE = 64.0              # W1 fp8 pre-scale

F32 = mybir.dt.float32
F16 = mybir.dt.float16
F8 = mybir.dt.float8e4
ALU = mybir.AluOpType
AF = mybir.ActivationFunctionType
AX = mybir.AxisListType
DR = mybir.MatmulPerfMode.DoubleRow


def _emit(ctx: ExitStack, tc: "tile.TileContext", out, tok, W, biasT_in, vvec):
    nc = tc.nc

    const = ctx.enter_context(tc.tile_pool(name="const", bufs=1))
    xres = ctx.enter_context(tc.tile_pool(name="xres", bufs=28))
    xtp = ctx.enter_context(tc.tile_pool(name="xtp", bufs=2))
    xbp = ctx.enter_context(tc.tile_pool(name="xbp", bufs=8))
    ep = ctx.enter_context(tc.tile_pool(name="ep", bufs=9))
    wsp = ctx.enter_context(tc.tile_pool(name="wsp", bufs=4))
    smp = ctx.enter_context(tc.tile_pool(name="smp", bufs=1))

    # PSUM is 8 banks, bank-granular: mm 4 + tp 3 + vd 1
    mm_ps = ctx.enter_context(tc.tile_pool(name="mm_ps", bufs=4, space="PSUM"))
    tp_ps = ctx.enter_context(tc.tile_pool(name="tp_ps", bufs=3, space="PSUM"))
    vd_ps = ctx.enter_context(tc.tile_pool(name="vd_ps", bufs=1, space="PSUM"))

    s_sb = const.tile([128, NT], F32)   # per-token-tile (1 + weight) scales

    # HAM warmup: dummy matmuls keep the PE busy from t=0 while the first
    # DMAs land, so the clock is at 8/8 when real work starts. The result
    # is copied into s_sb[:, 0:1], which is rewritten by the softmax
    # scales before any consumer reads it.
    warm = const.tile([128, 512], F8)
    nc.vector.memset(warm[:], 0.0)
    wps = None
    for r in range(16):
        wps = mm_ps.tile([128, 512], F32, tag="mm", name=f"warm_ps{r % 4}")
        nc.tensor.matmul(wps[:], warm[:, 0:128], warm[:],
                         start=True, stop=True, skip_group_check=True)
    nc.vector.tensor_copy(out=s_sb[:, 0:1], in_=wps[:, 0:1])

    # ---- constants / small inputs -------------------------------------
    ident = const.tile([128, 128], F32)
    make_identity(nc, ident[:])
    ident8 = const.tile([128, 128], F8)
    make_identity(nc, ident8[:])

    v_sb = const.tile([128, MC], F16)            # v[m*128+p] at [p, m]
    v_stg = const.tile([128, MC], F32)
    nc.scalar.dma_start(v_stg[:], vvec.rearrange("(m p) -> p m", p=128))
    nc.vector.tensor_copy(out=v_sb[:], in_=v_stg[:])

    biasT = const.tile([128, MC, BPC], F32)      # bias[b, m*128+p] at [p, m, b]
    nc.scalar.dma_start(biasT[:], biasT_in.rearrange("(m p) b -> p m b", p=128))

    # ---- main pipeline ------------------------------------------------
    tok_t = tok.rearrange("(n p) h -> n p h", p=128)
    out_t = out.rearrange("(n p) h -> n p h", p=128)

    x_nat = {}          # n -> resident [128, H] f32 tile
    sc_rows = {}        # t -> [1, 512] score row awaiting transpose
    sT = {}             # bb -> [128, 16] transposed scores
    for bb in range(BPC):
        sT[bb] = smp.tile([128, 16], F32, tag=f"sT{bb}", name=f"sT{bb}")

    def load_x(t):
        for j in range(4):
            n = 4 * t + j
            xt_ = xres.tile([128, H], F32, tag="x", name=f"x{n}")
            x_nat[n] = xt_
            if n >= 16 and n % 4 == 3:
                eng = nc.scalar
            else:
                eng = nc.sync if n % 2 == 0 else nc.gpsimd
            eng.dma_start(xt_[:], tok_t[n])

    def emit_score_transpose(t, jj):
        bb = t // (NT512 // BPC)
        row = sc_rows[t]
        col = 4 * (t % (NT512 // BPC)) + jj
        ps = tp_ps.tile([128, 1], F32, tag="tp")
        nc.tensor.transpose(
            ps[:], row[0:1, jj * 128 : (jj + 1) * 128], ident[0:1, 0:1])
        nc.vector.tensor_copy(out=sT[bb][:, col : col + 1], in_=ps[:])

    def emit_softmax(bb):
        # softmax over the transposed [128, 16] score block, then the
        # per-token scale s = 1 + exp(x - max)/sum
        stile = sT[bb]
        pmax = smp.tile([128, 1], F32, tag="pmax", name=f"pmax{bb}")
        nc.vector.tensor_reduce(pmax[:], stile[:], axis=AX.X, op=ALU.max)
        gmax = smp.tile([128, 1], F32, tag="gmax", name=f"gmax{bb}")
        nc.gpsimd.partition_all_reduce(
            gmax[:], pmax[:], channels=128, reduce_op=bass_isa.ReduceOp.max)
        negmax = smp.tile([128, 1], F32, tag="negmax", name=f"negmax{bb}")
        nc.vector.tensor_scalar(negmax[:], gmax[:], -1.0, None, op0=ALU.mult)
        acc = smp.tile([128, 1], F32, tag="acc", name=f"acc{bb}")
        sl = s_sb[:, bb * 16 : (bb + 1) * 16]
        nc.scalar.activation(sl, stile[:], AF.Exp, bias=negmax[:],
                             accum_out=acc[:])
        gsum = smp.tile([128, 1], F32, tag="gsum", name=f"gsum{bb}")
        nc.gpsimd.partition_all_reduce(
            gsum[:], acc[:], channels=128, reduce_op=bass_isa.ReduceOp.add)
        rc = smp.tile([128, 1], F32, tag="rc", name=f"rc{bb}")
        nc.vector.reciprocal(rc[:], gsum[:])
        nc.vector.tensor_scalar(sl, sl, rc[:], 1.0, op0=ALU.mult, op1=ALU.add)

    def emit_scales(bb, js, queues, use_act=False):
        for i, j in enumerate(js):
            n = bb * (NT // BPC) + j
            if use_act and i % 2 == 1:
                nc.scalar.mul(x_nat[n][:], x_nat[n][:], s_sb[:, n : n + 1])
            else:
                nc.vector.tensor_scalar(
                    x_nat[n][:], x_nat[n][:], s_sb[:, n : n + 1], None,
                    op0=ALU.mult)
            queues[i % len(queues)].dma_start(out_t[n], x_nat[n][:])

    def emit_casts(t):
        xbs = []
        for j in range(4):
            n = 4 * t + j
            xb = xbp.tile([128, H], F8, tag="xb", name=f"xb{n}")
            nc.vector.tensor_copy(out=xb[:], in_=x_nat[n][:])
            xbs.append(xb)
        return xbs

    def emit_transpose_pair(xbs, xT, kp):
        # 8 PE transposes (2 k-chunks x 4 token tiles) + one DVE copy.
        # fp8 transpose-mode writes PSUM with element step 2 (HW rule).
        ps = tp_ps.tile([128, 1024, 2], F8, tag="tp")
        for kk in range(2):
            k = 2 * kp + kk
            for j in range(4):
                nc.tensor.transpose(
                    ps[:, kk * 512 + j * 128 : kk * 512 + (j + 1) * 128, 0],
                    xbs[j][:, k * 128 : (k + 1) * 128], ident8[:])
        nc.vector.tensor_copy(out=xT[:, 2 * kp : 2 * kp + 2, :],
                              in_=ps[:, :, 0])

    # ---- startup DMAs: x0-3, then W1 (4MB) split over all 3 queues ----
    load_x(0)
    w1_sb = const.tile([128, KC, H], F8)         # 64*W1[k*128+p, m] at [p, k, m]
    w1_src = W[0:H, :].rearrange("(k p) m -> p k m", p=128)
    w1stg = {}
    for k in range(KC):
        stg = wsp.tile([128, H], F32, tag="w1stg", bufs=4, name=f"w1stg{k}")
        (nc.sync if k % 2 == 0 else nc.gpsimd).dma_start(stg[:], w1_src[:, k])
        w1stg[k] = stg
    load_x(1)
    load_x(2)

    def emit_w1_casts(ks):
        for k in ks:
            nc.vector.tensor_scalar(
                w1_sb[:, k, :], w1stg[k][:], WSCALE, None, op0=ALU.mult)

    # block 0's fp16 casts + transposes run while W1 streams in
    xT_cur = xtp.tile([128, KC, 512], F8, tag="xT", name="xT0")
    fill0 = vd_ps.tile([1, 512], F32, tag="vd", name="fill0")
    xbs0 = emit_casts(0)
    for kp in range(KP):
        emit_transpose_pair(xbs0, xT_cur, kp)
        for _ in range(3):
            nc.tensor.matmul(
                fill0[:], warm[:, 0:1], warm[:, 0:512],
                start=True, stop=True, skip_group_check=True)

    pending_fin = None
    pending_scale = None

    for t in range(NT512):
        bb = t // (NT512 // BPC)
        if t >= 1 and t + 2 < NT512:
            load_x(t + 2)

        # next block's casts (DVE) + its xT tile; transposes interleave
        # into this block's matmul stream below
        xT_next = None
        xbs_next = None
        if t + 1 < NT512:
            if t == 0:
                emit_w1_casts(range(0, 8))
            xbs_next = emit_casts(t + 1)
            xT_next = xtp.tile([128, KC, 512], F8, tag="xT", name=f"xT{t+1}")

        sc_ps = vd_ps.tile([1, 512], F32, tag="vd")
        e_tiles = []

        def tanh_group(m, mm):
            e = ep.tile([128, 512], F16, tag="e")
            nc.scalar.activation(e[:], mm[:], AF.Tanh,
                                 bias=biasT[:, m, bb : bb + 1],
                                 scale=1.0 / WSCALE)
            e_tiles.append((m, e))

        def after_group(m):
            # interleave next-block transposes and prev-block score
            # transposes between matmul groups: PE-transpose-mode does not
            # count as HAM activity, so bursts of them downclock the core
            if m % 2 == 1 and xbs_next is not None:
                emit_transpose_pair(xbs_next, xT_next, (m - 1) // 2)
            if m % 2 == 0 and t - 1 in sc_rows:
                emit_score_transpose(t - 1, m // 2)

        if t == 0:
            # k-pair-outer over four m-groups: matmuls start as W1 chunk
            # pairs arrive from HBM; m4-7 run k-pair-inner afterwards.
            mms = [mm_ps.tile([128, 512], F32, tag="mm", name=f"mm0_{m}")
                   for m in range(4)]
            fill = vd_ps.tile([1, 512], F32, tag="vd", name="fill")
            for kp in range(KP):
                for m in range(4):
                    nc.tensor.matmul(
                        mms[m][:],
                        w1_sb[:, 2 * kp : 2 * kp + 2, m * 128 : (m + 1) * 128],
                        xT_cur[:, 2 * kp : 2 * kp + 2, :],
                        start=(kp == 0), stop=(kp == KP - 1), perf_mode=DR)
                if kp < KP - 1:
                    for _ in range(4):
                        nc.tensor.matmul(
                            fill[:], warm[:, 0:1], warm[:, 0:512],
                            start=True, stop=True, skip_group_check=True)
            for m in range(4):
                tanh_group(m, mms[m])
                after_group(m)
            for m in range(4, MC):
                mm = mm_ps.tile([128, 512], F32, tag="mm")
                for i, kp in enumerate(range(KP)):
                    nc.tensor.matmul(
                        mm[:],
                        w1_sb[:, 2 * kp : 2 * kp + 2, m * 128 : (m + 1) * 128],
                        xT_cur[:, 2 * kp : 2 * kp + 2, :],
                        start=(i == 0), stop=(i == KP - 1), perf_mode=DR)
                tanh_group(m, mm)
                after_group(m)
        else:
            for m in range(MC):
                mm = mm_ps.tile([128, 512], F32, tag="mm")
                # group 0 reads k-pairs in reverse: its first matmul then
                # depends on the final transpose copy, which keeps the
                # scheduler from racing ahead of the transpose stream
                kps = list(reversed(range(KP))) if m == 0 else list(range(KP))
                for i, kp in enumerate(kps):
                    nc.tensor.matmul(
                        mm[:],
                        w1_sb[:, 2 * kp : 2 * kp + 2, m * 128 : (m + 1) * 128],
                        xT_cur[:, 2 * kp : 2 * kp + 2, :],
                        start=(i == 0), stop=(i == KP - 1), perf_mode=DR)
                tanh_group(m, mm)
                after_group(m)

        # batched v-dots: fp16 matmuls count as PE activity and run at
        # full rate back-to-back; one fp8<->fp16 mode switch per block
        for pm, pe_t in e_tiles:
            nc.tensor.matmul(
                sc_ps[:], v_sb[:, pm : pm + 1], pe_t[:],
                start=(pm == 0), stop=(pm == MC - 1), skip_group_check=True)
        row = smp.tile([1, 512], F32, tag="scrow", bufs=2, name=f"row{t}")
        nc.scalar.copy(row[:], sc_ps[:])
        if t - 1 in sc_rows:
            del sc_rows[t - 1]
        sc_rows[t] = row
        xT_cur = xT_next

        # finalize: softmax once a batch's scores are complete, then
        # scale+store spread 4 tiles per block to avoid engine bursts
        if pending_fin is not None:
            emit_softmax(pending_fin)
            pending_scale = (pending_fin, 0)
            pending_fin = None
        if pending_scale is not None:
            sb_, off = pending_scale
            emit_scales(sb_, range(off, off + 4), [nc.sync, nc.gpsimd],
                        use_act=True)
            pending_scale = (sb_, off + 4) if off + 4 < 16 else None

        if t % (NT512 // BPC) == (NT512 // BPC) - 1:
            pending_fin = bb

    # tail: remaining score transposes + last batch softmax/scales;
    # stores fan out over all three queues
    if pending_scale is not None:
        sb_, off = pending_scale
        emit_scales(sb_, range(off, 16), [nc.sync, nc.scalar])
    tlast = NT512 - 1
    for jj in range(4):
        emit_score_transpose(tlast, jj)
    emit_softmax(pending_fin)
    emit_scales(pending_fin, range(0, 16), [nc.sync, nc.scalar])


_CACHE = {}


def _build():
    if "nc" in _CACHE:
        return _CACHE["nc"]
    nc = bacc.Bacc("TRN2", target_bir_lowering=False, debug=False,
                   num_devices=NCORES)
    tok = nc.dram_tensor("tok", [T, H], F32, kind="ExternalInput").ap()
    W_ = nc.dram_tensor("W", [2 * H, H], F32, kind="ExternalInput").ap()
    bT = nc.dram_tensor("biasT", [H, BPC], F32, kind="ExternalInput").ap()
    v_ = nc.dram_tensor("v", [H], F32, kind="ExternalInput").ap()
    outp = nc.dram_tensor("out", [T, H], F32, kind="ExternalOutput").ap()

    with tile.TileContext(nc) as tc:
        with ExitStack() as ctx:
            _emit(ctx, tc, outp, tok, W_, bT, v_)
    nc.compile()
    _CACHE["nc"] = nc
    return nc


def host_bias(aspect_embedding, W, b):
    """bias[b, h] = mean_A(asp)[b] @ W2 + b  (64KB constant, on host)."""
    am = aspect_embedding.astype(np.float64).mean(axis=1)      # [B, H]
    return (am @ W.astype(np.float64)[H:] + b.astype(np.float64)).astype(
        np.float32)                                            # [B, H]


def make_in_maps(token_embeddings, aspect_embedding, W, b, v):
    bias = host_bias(aspect_embedding, W, b)
    in_maps = []
    for c in range(NCORES):
        in_maps.append({
            "tok": np.ascontiguousarray(
                token_embeddings[BPC * c : BPC * (c + 1)].reshape(T, H)),
            "biasT": np.ascontiguousarray(
                bias[BPC * c : BPC * (c + 1)].T),              # [H, BPC]
            "W": W, "v": v,
        })
    return in_maps


def kernel(token_embeddings, aspect_embedding, W, b, v):
    token_embeddings = np.asarray(token_embeddings, dtype=np.float32)
    aspect_embedding = np.asarray(aspect_embedding, dtype=np.float32)
    W = np.asarray(W, dtype=np.float32)
    b = np.asarray(b, dtype=np.float32)
    v = np.asarray(v, dtype=np.float32)

    nc = _build()
    in_maps = make_in_maps(token_embeddings, aspect_embedding, W, b, v)
    res = run_bass_kernel_spmd(nc, in_maps, core_ids=list(range(NCORES)))
    return np.concatenate(
        [res.results[c]["out"].reshape(BPC, S, H) for c in range(NCORES)], axis=0)
